# revision 16
# baseline (speedup 1.0000x reference)
"""Trainium2 Bass kernel for nn_ConvNL (conv3x3+BN+ReLU -> NL1D attention -> BN+SiLU).

Sharding: data-parallel over batch B=16 across 8 NeuronCores (2 batches/core).
BatchNorm batch stats are synchronized with two tiny AllReduces ([128,2] f32).

Per-core pipeline (single NEFF):
  A) conv3x3 (reflect-padded on host, fp16) as 9 accumulating K=64 matmuls per
     512-elem output block; both local batches run concurrently on the PE via
     row tiling. PSUM blocks are copied to a resident fp16 h buffer while
     per-channel sum / sum-of-squares partials accumulate for BN1.
  B) AllReduce BN1 stats; u = relu(h_raw + c1) in place via 4x-mode
     tensor_scalar; row sums (xm) via 64 accumulating identity matmuls on the
     otherwise-idle PE; sum(u^2) for BN2 split between ACT (Square+accum) and
     DVE (tensor_tensor_reduce).
  C) Per batch: LN stats via DVE reductions + ones-matmul partition reduce,
     then a k=1 ones-matmul broadcast (no DRAM round trip); attention
     E = exp(S/sqrt(C) - 12) fp16; softmax denom via ones-matmul; denom
     reciprocal via reciprocal_approx_fast, broadcast by k=1 matmul;
     o = out_w z (+ b_eff kept separate); o^T produced directly by extra
     transposed matmuls into partitions 0-31 for phase D. BN2 stats
     analytically from xm, o, sum(u^2).
  D) AllReduce BN2 stats; t = a1*u + o built in PSUM by the PE (identity
     matmul scaled by diag(a1) + an o-paint matmul against a replicated-eye
     constant); single ACT pass silu(a2*t + (a2*b_eff + b2)) emits fp16
     straight to DRAM.
"""
import sys

sys.path.insert(0, "/opt/trn_rl_repo")

import numpy as np

import concourse.bass as bass
import concourse.tile as tile
from concourse import mybir
from concourse.bass_utils import run_bass_kernel_spmd

N_CORES = 8
B, CIN, W, C = 16, 64, 64, 128
BPC = B // N_CORES  # batches per core
WP = W + 2
EPS = 1e-5

f16, f32 = mybir.dt.float16, mybir.dt.float32
AX = mybir.AxisListType
OP = mybir.AluOpType
AF = mybir.ActivationFunctionType
CORE_IDS = list(range(N_CORES))


def _split_syncwaits(nc, max_waits=1):
    """This walrus build rejects instructions with more than a couple of
    sync-wait commands; split excess waits onto InstDrain carriers."""
    for f in nc.m.functions:
        for bb in f.blocks:
            new_insts = []
            for inst in bb.instructions:
                si = inst.sync_info
                waits = list(si.on_wait) if si and si.on_wait else []
                if len(waits) > max_waits:
                    head, tail = waits[:-max_waits], waits[-max_waits:]
                    while head:
                        chunk, head = head[:max_waits], head[max_waits:]
                        carrier = mybir.InstDrain(
                            name=f"I-waitsplit-{nc.next_id()}",
                            ins=[], outs=[], engine=inst.engine,
                        )
                        carrier.sync_info = mybir.SyncInfo(on_wait=chunk, on_update=[])
                        new_insts.append(carrier)
                    inst.sync_info = mybir.SyncInfo(
                        on_wait=tail,
                        on_update=list(si.on_update) if si.on_update else [],
                    )
                new_insts.append(inst)
            bb.instructions[:] = new_insts


def _allreduce2(nc, dram_pool, src2, dst2, local_cc, tag):
    """AllReduce a [128,2] f32 stat tile across the 8 cores (sum)."""
    ar_in = dram_pool.tile([128, 2], f32, name=f"arin_{tag}")
    nc.sync.dma_start(out=ar_in, in_=src2)
    if local_cc:
        nc.sync.dma_start(out=dst2, in_=ar_in)
        return
    ar_out = dram_pool.tile([128, 2], f32, addr_space="Shared", name=f"arout_{tag}")
    nc.gpsimd.collective_compute(
        "AllReduce", OP.add,
        replica_groups=[CORE_IDS],
        ins=[ar_in.opt()], outs=[ar_out.opt()],
    )
    nc.sync.dma_start(out=dst2, in_=ar_out)


def _bn_coeffs(nc, pool, sums2, g_ap, b_ap, n_tot, eps_t, tag):
    """From AllReduced [sum, sumsq] (cols of sums2) compute the BN affine:
    a = g*rstd, bshift = b - mu*a. Returns (a, bshift, mu)."""
    mu = pool.tile([128, 1], f32, name=f"mu_{tag}")
    nc.vector.tensor_scalar_mul(out=mu, in0=sums2[:, 0:1], scalar1=1.0 / n_tot)
    ex2 = pool.tile([128, 1], f32, name=f"ex2_{tag}")
    nc.vector.tensor_scalar_mul(out=ex2, in0=sums2[:, 1:2], scalar1=1.0 / n_tot)
    nmu2 = pool.tile([128, 1], f32, name=f"nmu2_{tag}")
    nc.vector.tensor_scalar(out=nmu2, in0=mu, scalar1=mu, scalar2=-1.0,
                            op0=OP.mult, op1=OP.mult)
    var = pool.tile([128, 1], f32, name=f"var_{tag}")
    nc.vector.tensor_add(out=var, in0=ex2, in1=nmu2)
    sd = pool.tile([128, 1], f32, name=f"sd_{tag}")
    nc.scalar.activation(out=sd, in_=var, func=AF.Sqrt, bias=eps_t, scale=1.0)
    rstd = pool.tile([128, 1], f32, name=f"rstd_{tag}")
    nc.vector.reciprocal(out=rstd, in_=sd)
    a = pool.tile([128, 1], f32, name=f"a_{tag}")
    nc.vector.tensor_mul(out=a, in0=g_ap, in1=rstd)
    mua = pool.tile([128, 1], f32, name=f"mua_{tag}")
    nc.vector.tensor_mul(out=mua, in0=mu, in1=a)
    bshift = pool.tile([128, 1], f32, name=f"bsh_{tag}")
    nc.vector.tensor_sub(out=bshift, in0=b_ap, in1=mua)
    return a, bshift, mu


def _kernel(ctx, tc, xp, wt, gw, ow, pars, eye, eyew, out, H, local_cc):
    nc = tc.nc
    NCHUNK = H // 64
    NBLK = H // 8          # per batch, 8 output rows (512 elems) per block
    MI = H // 128          # attention M-chunks
    NSQ = 32               # 1024-elem square chunks per batch
    SQ_ACT = 20            # chunks squared on ACT; rest on DVE
    n_tot = float((BPC if local_cc else B) * H * W)
    n_ln = float(C * H)

    consts = ctx.enter_context(tc.tile_pool(name="consts", bufs=1))
    big = ctx.enter_context(tc.tile_pool(name="big", bufs=1))
    stats = ctx.enter_context(tc.tile_pool(name="stats", bufs=1))
    dram = ctx.enter_context(tc.tile_pool(name="dram", bufs=1, space="DRAM"))

    wt_sb = consts.tile([128, 9, 128], f16)
    nc.sync.dma_start(out=wt_sb, in_=wt)
    gw_sb = consts.tile([128, 128], f16)
    nc.sync.dma_start(out=gw_sb, in_=gw)
    ow_sb = consts.tile([128, 128], f16)
    nc.sync.dma_start(out=ow_sb, in_=ow)
    pars_sb = consts.tile([128, 8], f32)
    nc.sync.dma_start(out=pars_sb, in_=pars)
    eye_sb = consts.tile([128, 128], f16)
    nc.sync.dma_start(out=eye_sb, in_=eye)
    eyew_sb = consts.tile([32, 2048], f16)
    nc.sync.dma_start(out=eyew_sb, in_=eyew)
    ones16 = consts.tile([128, 1], f16)
    nc.vector.memset(ones16, 1.0)
    ones32 = consts.tile([128, 1], f32)
    nc.vector.memset(ones32, 1.0)
    onesr = consts.tile([1, 128], f32)
    nc.vector.memset(onesr, 1.0)
    eps_t = consts.tile([128, 1], f32)
    nc.vector.memset(eps_t, EPS)
    shift_t = consts.tile([128, 1], f32)
    nc.vector.memset(shift_t, -12.0)

    h_sb = big.tile([128, BPC, H * W], f16)

    s1_acc = stats.tile([128, BPC * NBLK], f32)
    s2_acc = stats.tile([128, BPC * NBLK // 2], f32)
    r2acc = stats.tile([128, BPC * NSQ], f32)
    xms = stats.tile([128, BPC, H], f32)
    o_all = stats.tile([128, BPC, H], f32)
    oT32 = stats.tile([32, BPC, 16, 128], f16)
    s1b = stats.tile([128, BPC], f32)
    s2ob = stats.tile([128, BPC], f32)
    star1 = stats.tile([128, 2], f32)
    star2 = stats.tile([128, 2], f32)
    diag_a1 = stats.tile([128, 128], f16)

    # ---------------- Phase A: conv + BN1 partials ----------------
    with tc.tile_pool(name="xinp", bufs=2) as xinp, \
         tc.tile_pool(name="scrA", bufs=2) as scrA, \
         tc.tile_pool(name="psA", bufs=3, space="PSUM") as psA:
        for ch in range(NCHUNK):
            xin = xinp.tile([128, 66, WP], f16)
            nc.sync.dma_start(out=xin, in_=xp[:, ch * 64 * WP: (ch * 64 + 66) * WP])
            for j in range(8):
                ps = [psA.tile([128, 512], f32, name=f"ps{b}") for b in range(BPC)]
                for t in range(9):
                    dy, dx = t // 3, t % 3
                    r0 = 8 * j + dy
                    for b in range(BPC):
                        nc.tensor.matmul(
                            ps[b],
                            lhsT=wt_sb[b * 64:(b + 1) * 64, t, :],
                            rhs=xin[b * 64:(b + 1) * 64, r0:r0 + 8, dx:dx + W],
                            start=(t == 0), stop=(t == 8),
                        )
                blk = ch * 8 + j
                for b in range(BPC):
                    col = b * NBLK + blk
                    hv = h_sb[:, b, blk * 512:(blk + 1) * 512]
                    nc.vector.tensor_scalar(
                        out=hv, in0=ps[b], scalar1=1.0, scalar2=0.0,
                        op0=OP.mult, op1=OP.add,
                        accum_out=s1_acc[:, col:col + 1])
                if j % 2 == 1:
                    for b in range(BPC):
                        col = b * (NBLK // 2) + blk // 2
                        hv2 = h_sb[:, b, (blk - 1) * 512:(blk + 1) * 512]
                        scr = scrA.tile([128, 1024], f16, name="scr")
                        nc.scalar.activation(
                            out=scr, in_=hv2, func=AF.Square,
                            accum_out=s2_acc[:, col:col + 1])

    # ---------------- BN1 finalize ----------------
    s1v = stats.tile([128, 1], f32)
    nc.vector.reduce_sum(out=s1v, in_=s1_acc, axis=AX.X)
    s2v = stats.tile([128, 1], f32)
    nc.vector.reduce_sum(out=s2v, in_=s2_acc, axis=AX.X)
    st2 = stats.tile([128, 2], f32)
    nc.vector.tensor_copy(out=st2[:, 0:1], in_=s1v)
    nc.vector.tensor_copy(out=st2[:, 1:2], in_=s2v)
    _allreduce2(nc, dram, st2, star1, local_cc, "bn1")
    a1, b1s, mu1 = _bn_coeffs(nc, stats, star1, pars_sb[:, 0:1],
                              pars_sb[:, 1:2], n_tot, eps_t, "bn1")
    # c1 = b1s/a1 (a1 > 0 assumed)
    ra1 = stats.tile([128, 1], f32)
    nc.vector.reciprocal(out=ra1, in_=a1)
    c1 = stats.tile([128, 1], f32)
    nc.vector.tensor_mul(out=c1, in0=b1s, in1=ra1)
    nc.vector.tensor_scalar(out=diag_a1, in0=eye_sb, scalar1=a1, scalar2=None,
                            op0=OP.mult)

    # ---------- Phase B + C ----------
    with tc.tile_pool(name="attn", bufs=1) as attn, \
         tc.tile_pool(name="scra", bufs=2) as scra, \
         tc.tile_pool(name="scrv", bufs=2) as scrv, \
         tc.tile_pool(name="psB", bufs=1, space="PSUM") as psBp, \
         tc.tile_pool(name="psC", bufs=1, space="PSUM") as psCp:
        # u = relu(h + c1) in place; 4x-mode tensor_scalar (fp16, SBUF)
        for b in range(BPC):
            for un in range(8):
                hv = h_sb[:, b, un * 4096:(un + 1) * 4096]
                nc.vector.tensor_scalar(out=hv, in0=hv, scalar1=c1,
                                        scalar2=0.0, op0=OP.add, op1=OP.max)

        psXm = [None, None]

        def emit_rowsum(b, w_lo, w_hi):
            u3 = h_sb[:, b, :].rearrange("p (h w) -> p h w", w=W)
            for j in range(w_lo, w_hi):
                nc.tensor.matmul(psXm[b], lhsT=eye_sb, rhs=u3[:, :, j],
                                 start=(j == 0), stop=(j == W - 1))

        def emit_squares(b, engine):
            # ACT chunks first (engine='a'), DVE chunks ('v')
            rng = range(SQ_ACT) if engine == 'a' else range(SQ_ACT, NSQ)
            for sq in rng:
                col = b * NSQ + sq
                uv = h_sb[:, b, sq * 1024:(sq + 1) * 1024]
                if engine == 'a':
                    scr = scra.tile([128, 1024], f16, name="scr")
                    nc.scalar.activation(out=scr, in_=uv, func=AF.Square,
                                         accum_out=r2acc[:, col:col + 1])
                else:
                    scr = scrv.tile([128, 1024], f16, name="scr")
                    nc.vector.scalar_tensor_tensor(
                        out=scr, in0=uv, scalar=1.0, in1=uv,
                        op0=OP.mult, op1=OP.mult,
                        accum_out=r2acc[:, col:col + 1])

        def emit_ln(b):
            xmsv = xms[:, b, :]
            nc.vector.tensor_scalar(out=xmsv, in0=psXm[b], scalar1=a1,
                                    scalar2=1.0 / W, op0=OP.mult, op1=OP.mult)
            rsum = attn.tile([128, 1], f32, name="rsum")
            nc.vector.reduce_sum(out=rsum, in_=xmsv, axis=AX.X)
            rsq = attn.tile([128, 1], f32, name="rsq")
            scr32 = attn.tile([128, H], f32, name="scr32")
            nc.vector.scalar_tensor_tensor(
                out=scr32, in0=xmsv, scalar=1.0, in1=xmsv,
                op0=OP.mult, op1=OP.mult, accum_out=rsq)
            sin = attn.tile([128, 2], f32, name="sin")
            nc.vector.tensor_copy(out=sin[:, 0:1], in_=rsum)
            nc.vector.tensor_copy(out=sin[:, 1:2], in_=rsq)
            psLN = psCp.tile([128, 2], f32, name="psln2")
            nc.tensor.matmul(psLN[0:1, :], lhsT=ones32, rhs=sin,
                             start=True, stop=True)
            tots = attn.tile([1, 2], f32, name="tots")
            nc.vector.tensor_copy(out=tots, in_=psLN[0:1, :])
            psLB = psCp.tile([128, 2], f32, name="psln2")
            nc.tensor.matmul(psLB, lhsT=onesr, rhs=tots, start=True, stop=True)
            muv = attn.tile([128, 1], f32, name="muv")
            nc.vector.tensor_scalar_mul(out=muv, in0=psLB[:, 0:1],
                                        scalar1=1.0 / n_ln)
            ex2v = attn.tile([128, 1], f32, name="ex2v")
            nc.vector.tensor_scalar_mul(out=ex2v, in0=psLB[:, 1:2],
                                        scalar1=1.0 / n_ln)
            nmu2v = attn.tile([128, 1], f32, name="nmu2v")
            nc.vector.tensor_scalar(out=nmu2v, in0=muv, scalar1=muv,
                                    scalar2=-1.0, op0=OP.mult, op1=OP.mult)
            varv = attn.tile([128, 1], f32, name="varv")
            nc.vector.tensor_add(out=varv, in0=ex2v, in1=nmu2v)
            sdv = attn.tile([128, 1], f32, name="sdv")
            nc.scalar.activation(out=sdv, in_=varv, func=AF.Sqrt,
                                 bias=eps_t, scale=1.0)
            rstdv = attn.tile([128, 1], f32, name="rstdv")
            nc.vector.reciprocal(out=rstdv, in_=sdv)
            xn16 = attn.tile([128, H], f16, name="xn16")
            nc.vector.tensor_scalar(out=xn16, in0=xmsv, scalar1=muv,
                                    scalar2=rstdv, op0=OP.subtract,
                                    op1=OP.mult)
            return xn16

        def emit_attn(b, xn16):
            # S = xn^T xn; E = exp(S/sqrt(C) - 12) fp16
            E16 = attn.tile([128, MI, H], f16, name="E16")
            for mi in range(MI):
                psS = psCp.tile([128, H], f32, name="psS")
                nc.tensor.matmul(psS, lhsT=xn16[:, mi * 128:(mi + 1) * 128],
                                 rhs=xn16, start=True, stop=True)
                nc.scalar.activation(out=E16[:, mi, :], in_=psS, func=AF.Exp,
                                     scale=float(1.0 / np.sqrt(C)), bias=shift_t)
            # denom[h] = sum_k E[k,h]; reciprocal; broadcast via k=1 matmul
            psDn = psCp.tile([128, H], f32, name="psmisc")
            for mi in range(MI):
                nc.tensor.matmul(psDn[0:1, :], lhsT=ones16, rhs=E16[:, mi, :],
                                 start=(mi == 0), stop=(mi == MI - 1))
            dvec = attn.tile([1, H], f32, name="dvec")
            nc.vector.tensor_copy(out=dvec, in_=psDn[0:1, :])
            rvec = attn.tile([1, H], f32, name="rvec")
            nc.vector.reciprocal(out=rvec, in_=dvec)
            psR = psCp.tile([128, H], f32, name="psmisc")
            nc.tensor.matmul(psR, lhsT=onesr, rhs=rvec, start=True, stop=True)
            rb = attn.tile([128, H], f32, name="rb")
            nc.vector.tensor_copy(out=rb, in_=psR)
            # yT[k,m] = sum_c xn[c,k] gw[m,c]
            yT16 = attn.tile([128, MI, 128], f16, name="yT16")
            for mi in range(MI):
                psY = psCp.tile([128, 128], f32, name="psY")
                nc.tensor.matmul(psY, lhsT=xn16[:, mi * 128:(mi + 1) * 128],
                                 rhs=gw_sb, start=True, stop=True)
                nc.vector.tensor_copy(out=yT16[:, mi, :], in_=psY)
            # z[m,h] = (sum_k yT[k,m] E[k,h]) / denom[h]
            psZ = psCp.tile([128, H], f32, name="pszx")
            for mi in range(MI):
                nc.tensor.matmul(psZ, lhsT=yT16[:, mi, :], rhs=E16[:, mi, :],
                                 start=(mi == 0), stop=(mi == MI - 1))
            z16 = attn.tile([128, H], f16, name="z16")
            nc.vector.tensor_mul(out=z16, in0=psZ, in1=rb)
            # o (channel-major, + b_eff) for BN2 stats
            psX = psCp.tile([128, H], f32, name="pszx")
            nc.tensor.matmul(psX, lhsT=ow_sb, rhs=z16, start=True, stop=True)
            ov = o_all[:, b, :]
            nc.vector.tensor_scalar_add(out=ov, in0=psX, scalar1=pars_sb[:, 4:5])
            # oT (h-major on partitions 0-31, WITHOUT b_eff) for phase D paint
            for g in range(4):
                psT = psCp.tile([32, 512], f32, name="psT")
                for s in range(4):
                    sl = g * 4 + s
                    nc.tensor.matmul(psT[:, s * 128:(s + 1) * 128],
                                     lhsT=z16[:, sl * 32:(sl + 1) * 32],
                                     rhs=ow_sb, start=True, stop=True)
                nc.vector.tensor_copy(
                    out=oT32[:, b, g * 4:(g + 1) * 4, :],
                    in_=psT.rearrange("p (a c) -> p a c", c=128))
            # BN2 partials: sum_w t = W*(xm + o); sum t^2 uses o*(2xm+o)
            xmsv = xms[:, b, :]
            tmp1 = attn.tile([128, H], f32, name="tmp1")
            nc.vector.scalar_tensor_tensor(out=tmp1, in0=ov, scalar=1.0,
                                           in1=xmsv, op0=OP.mult, op1=OP.add,
                                           accum_out=s1b[:, b:b + 1])
            tmp2 = attn.tile([128, H], f32, name="tmp2")
            nc.vector.tensor_scalar(out=tmp2, in0=xmsv, scalar1=2.0,
                                    scalar2=None, op0=OP.mult)
            tmp3 = attn.tile([128, H], f32, name="tmp3")
            nc.vector.tensor_tensor(out=tmp3, in0=tmp2, in1=ov, op=OP.add)
            scr32b = attn.tile([128, H], f32, name="scr32b")
            nc.vector.scalar_tensor_tensor(out=scr32b, in0=ov, scalar=1.0,
                                           in1=tmp3, op0=OP.mult, op1=OP.mult,
                                           accum_out=s2ob[:, b:b + 1])

        # emission order tuned for per-engine queues
        psXm[0] = psBp.tile([128, H], f32, name="psXm0")
        psXm[1] = psBp.tile([128, H], f32, name="psXm1")
        emit_rowsum(0, 0, W)
        emit_squares(0, 'a')
        emit_squares(0, 'v')
        emit_rowsum(1, 0, W // 2)
        xn0 = emit_ln(0)
        emit_attn(0, xn0)
        emit_rowsum(1, W // 2, W)
        emit_squares(1, 'a')
        emit_squares(1, 'v')
        xn1 = emit_ln(1)
        emit_attn(1, xn1)

    # ---------------- BN2 finalize ----------------
    a1sq = stats.tile([128, 1], f32)
    nc.vector.tensor_mul(out=a1sq, in0=a1, in1=a1)
    r2s = stats.tile([128, 1], f32)
    nc.vector.reduce_sum(out=r2s, in_=r2acc, axis=AX.X)
    s1s = stats.tile([128, 1], f32)
    nc.vector.reduce_sum(out=s1s, in_=s1b, axis=AX.X)
    s2os = stats.tile([128, 1], f32)
    nc.vector.reduce_sum(out=s2os, in_=s2ob, axis=AX.X)
    st2b = stats.tile([128, 2], f32)
    nc.vector.tensor_scalar_mul(out=st2b[:, 0:1], in0=s1s, scalar1=float(W))
    # S2 = a1^2 * sum(u^2) + W * sum(o*(2xm+o))
    tmp4 = stats.tile([128, 1], f32)
    nc.vector.tensor_scalar_mul(out=tmp4, in0=s2os, scalar1=float(W))
    tmp5 = stats.tile([128, 1], f32)
    nc.vector.tensor_mul(out=tmp5, in0=r2s, in1=a1sq)
    nc.vector.tensor_add(out=st2b[:, 1:2], in0=tmp5, in1=tmp4)
    _allreduce2(nc, dram, st2b, star2, local_cc, "bn2")
    a2, b2s, _ = _bn_coeffs(nc, stats, star2, pars_sb[:, 2:3],
                            pars_sb[:, 3:4], n_tot, eps_t, "bn2")
    # silu bias: a2*b_eff + b2s (b_eff excluded from the oT paint)
    bias_d = stats.tile([128, 1], f32)
    nc.vector.tensor_scalar(out=bias_d, in0=pars_sb[:, 4:5], scalar1=a2,
                            scalar2=b2s, op0=OP.mult, op1=OP.add)

    # ------ Phase D: t = a1*u + o in PSUM (PE); out = silu(a2*t + bias_d) ------
    with tc.tile_pool(name="outp", bufs=3) as outp, \
         tc.tile_pool(name="psD", bufs=2, space="PSUM") as psDp:
        for b in range(BPC):
            for chv in range(16):
                ps = psDp.tile([128, 2048], f32, name="psd")
                for s in range(4):
                    blk = ps[:, s * 512:(s + 1) * 512]
                    nc.tensor.matmul(blk, lhsT=oT32[:, b, chv, :],
                                     rhs=eyew_sb[:, s * 512:(s + 1) * 512],
                                     start=True, stop=False)
                    h0 = (chv * 32 + s * 8) * 64
                    nc.tensor.matmul(blk, lhsT=diag_a1,
                                     rhs=h_sb[:, b, h0:h0 + 512],
                                     start=False, stop=True)
                ot = outp.tile([128, 2048], f16, name="ot")
                nc.scalar.activation(out=ot, in_=ps, func=AF.Silu,
                                     scale=a2, bias=bias_d)
                nc.sync.dma_start(
                    out=out[b, :, chv * 32:(chv + 1) * 32, :],
                    in_=ot.rearrange("p (h w) -> p h w", w=W))


def build(H=512, local_cc=False):
    nc = bass.Bass("TRN2", target_bir_lowering=False, debug=False,
                   num_devices=N_CORES)
    HP = H + 2
    xp = nc.dram_tensor("xp", [128, HP * WP], f16, kind="ExternalInput").ap()
    wt = nc.dram_tensor("wt", [128, 9, 128], f16, kind="ExternalInput").ap()
    gw = nc.dram_tensor("gw", [128, 128], f16, kind="ExternalInput").ap()
    ow = nc.dram_tensor("ow", [128, 128], f16, kind="ExternalInput").ap()
    pars = nc.dram_tensor("pars", [128, 8], f32, kind="ExternalInput").ap()
    eye = nc.dram_tensor("eye", [128, 128], f16, kind="ExternalInput").ap()
    eyew = nc.dram_tensor("eyew", [32, 2048], f16, kind="ExternalInput").ap()
    out = nc.dram_tensor("out", [BPC, C, H, W], f16, kind="ExternalOutput").ap()
    from contextlib import ExitStack

    with tile.TileContext(nc) as tc:
        with ExitStack() as ctx:
            _kernel(ctx, tc, xp, wt, gw, ow, pars, eye, eyew, out, H, local_cc)
    _split_syncwaits(nc)
    return nc


def prep_inputs(x, conv_w, bn1_g, bn1_b, g_w, g_b, out_w, out_b, bn2_g, bn2_b):
    x = np.asarray(x, np.float32)
    conv_w = np.asarray(conv_w, np.float32)
    g_w = np.asarray(g_w, np.float32)
    out_w = np.asarray(out_w, np.float32)
    n_cores = x.shape[0] // BPC
    xpad = np.pad(x, ((0, 0), (0, 0), (1, 1), (1, 1)), mode="reflect")
    xpad = xpad.astype(np.float16)
    hp = x.shape[2] + 2
    # [9, ci, co] -> duplicate ci across partition halves -> [p, 9, co]
    wt9 = conv_w.transpose(2, 3, 1, 0).reshape(9, CIN, C)
    wt9 = np.concatenate([wt9, wt9], axis=1).transpose(1, 0, 2)
    wt9 = np.ascontiguousarray(wt9, dtype=np.float16)
    gwT = np.ascontiguousarray(g_w.T, dtype=np.float16)
    owT = np.ascontiguousarray(out_w.T, dtype=np.float16)
    b_eff = out_w @ np.asarray(g_b, np.float32) + np.asarray(out_b, np.float32)
    pars = np.zeros((128, 8), np.float32)
    pars[:, 0] = bn1_g
    pars[:, 1] = bn1_b
    pars[:, 2] = bn2_g
    pars[:, 3] = bn2_b
    pars[:, 4] = b_eff
    eye = np.eye(128, dtype=np.float16)
    eyew = np.repeat(np.eye(32, dtype=np.float16)[:, :, None], W, axis=2)
    eyew = np.ascontiguousarray(eyew.reshape(32, 32 * W))
    in_maps = []
    for i in range(n_cores):
        xc = xpad[BPC * i: BPC * (i + 1)].reshape(128, hp * WP)
        in_maps.append({"xp": np.ascontiguousarray(xc), "wt": wt9, "gw": gwT,
                        "ow": owT, "pars": pars, "eye": eye, "eyew": eyew})
    return in_maps


_NC_CACHE = {}


def run(inputs, trace=False, tmpdir=None):
    if "full" not in _NC_CACHE:
        _NC_CACHE["full"] = build()
    nc = _NC_CACHE["full"]
    in_maps = prep_inputs(**inputs)
    res = run_bass_kernel_spmd(nc, in_maps, CORE_IDS, trace=trace, tmpdir=tmpdir)
    out = np.concatenate([res.results[i]["out"] for i in range(N_CORES)], axis=0)
    return out.astype(np.float32), res


def kernel(**inputs):
    out, _ = run(inputs)
    return out
